# revision 1
# baseline (speedup 1.0000x reference)
"""Trainium2 Bass kernel for nn_MemorizingTransformer (retrieval_knn).

Sharding: 8 cores = 2 batches x 4 head-pairs. Each core computes attention for
its batch and 2 heads plus its slice of the output projection; the host sums
the 4 partial outputs per batch (the "all-reduce after to_out").

Per-core algorithm (n=2048 tokens, dh=64, 2 heads, kret=32 memories):
  - q/k/v projected with weights stationary (fp16) -> qT/kT/vT in PSUM;
    a row-wise pass l2-normalizes q (folding in the logit scale) and k, and
    produces fp16 qhatT / khatT plus fp16 qhat rows and bf16 v rows.
  - local attention computed TRANSPOSED (keys on partitions): one matmul per
    (key-tile, query-chunk) gives simT; since qhat/khat are unit-scaled the
    logits are bounded by the scale, so exp uses a constant shift and its
    bf16 output is multiplied by a host-precomputed exp(rel_pos_bias) table
    (transposed/blocked, causal zeros) to give attention weights already in
    the [key, query] layout the PV matmul needs as its stationary operand.
  - an extra ones-column of v gives the local softmax partition sum for free.
  - memory attention (per-query kNN keys/values) runs row-major on
    GPSIMD (broadcast products) + DVE (two-stage 16-bit axis reductions);
    a ones-row of memv gives the memory partition sum.
  - the two branches use different softmax shifts; the combine step rescales
    with a = exp(Cm-C*), b = exp(Cl-C*), C* = max(Cm, Cl) (both <= 1, no
    overflow/underflow blowups) before the output projection.
  - stream pools are allocated below the phase-A pools so phase-B DMA
    prefetch overlaps the projections.
"""

import numpy as np
import ml_dtypes
from contextlib import ExitStack

import concourse.bass as bass
import concourse.bacc as bacc
import concourse.mybir as mybir
import concourse.tile as tile
from concourse.masks import make_identity

F32 = mybir.dt.float32
BF16 = mybir.dt.bfloat16
F16 = mybir.dt.float16
AX = mybir.AxisListType
OP = mybir.AluOpType
ACTF = mybir.ActivationFunctionType

P = 128
DIM = 512
DH = 64
KRET = 32
HPC = 2            # heads per core
NCORES = 8
MASK_NEG = -1e30
C_LOC = 20.6       # >= scale * max|cos|: local exp args stay <= ~0
C_MEM = 60.0       # fixed mem-branch shift; mem logits ~N(0,20*|x|_dot) stay
                   # well under exp overflow after -C_MEM (max arg ~ +45)
B_LOC = float(np.exp(C_LOC - C_MEM))   # rescale for local sums in combine


def tree_add(nc, pool, prod, rows, width, out_f32, tag):
    """Sum prod[P, rows, width] over the last axis into out_f32 [P, rows]
    via binary-tree 16-bit tensor_tensor adds (DVE 2x mode) + f32 final."""
    cur = prod
    w = width
    with nc.allow_low_precision(reason="16-bit tree partial sums"):
        while w > 2:
            nxt = pool.tile([P, rows, w // 2], cur.dtype, tag=f"{tag}{w}",
                            name=f"{tag}{w}")
            nc.vector.tensor_tensor(out=nxt, in0=cur[:, :, 0:w // 2],
                                    in1=cur[:, :, w // 2:w], op=OP.add)
            cur = nxt
            w //= 2
    nc.vector.tensor_tensor(out=out_f32, in0=cur[:, :, 0],
                            in1=cur[:, :, 1], op=OP.add)


def bcast_mid(ap_2d, count):
    """[P, d] AP -> [P, count, d] AP broadcasting a new middle dim (step 0)."""
    return bass.AP(tensor=ap_2d.tensor, offset=ap_2d.offset,
                   ap=[list(ap_2d.ap[0]), [0, count], list(ap_2d.ap[1])])


def build_nc(n=2048):
    """Build the per-core Bass program (same NEFF for all 8 cores)."""
    nt = n // P               # 128-token tiles
    nq = n // 512             # 512-query chunks
    nc = bacc.Bacc("TRN2", target_bir_lowering=False, debug=False)

    xt_d = nc.dram_tensor("xt", (DIM, n), F16, kind="ExternalInput").ap()
    wq_d = nc.dram_tensor("wq", (DIM, HPC * DH), F16, kind="ExternalInput").ap()
    wkv_d = nc.dram_tensor("wkv", (DIM, 2 * DH), F16, kind="ExternalInput").ap()
    wout_d = nc.dram_tensor("wout", (HPC * DH, DIM), F16, kind="ExternalInput").ap()
    scales_d = nc.dram_tensor("scales", (1, HPC), F32, kind="ExternalInput").ap()
    memk_d = nc.dram_tensor("memk", (HPC, n, KRET, DH), F16, kind="ExternalInput").ap()
    memv_d = nc.dram_tensor("memv", (HPC, n, DH + 1, KRET), BF16, kind="ExternalInput").ap()
    mask_d = nc.dram_tensor("maskadd", (HPC, n, KRET), F32, kind="ExternalInput").ap()
    # expbias[h, c, j, i'] = exp(bias[h, 512c+i', j]) * (512c+i' >= j)
    expb_d = nc.dram_tensor("expbias", (HPC, nq, n, 512), BF16, kind="ExternalInput").ap()
    out_d = nc.dram_tensor("out", (n, DIM), F16, kind="ExternalOutput").ap()

    with tile.TileContext(nc) as tc, ExitStack() as ctx:
        persist = ctx.enter_context(tc.tile_pool(name="persist", bufs=1))

        # ---- constants -------------------------------------------------
        id_f = persist.tile([P, P], F32)
        make_identity(nc, id_f)
        id_h = persist.tile([P, P], F16)
        make_identity(nc, id_h)
        scales_sb = persist.tile([P, HPC], F32)
        nc.sync.dma_start(out=scales_sb, in_=bass.AP(
            tensor=scales_d.tensor, offset=scales_d.offset,
            ap=[[0, P], list(scales_d.ap[1])]))
        wout_sb = persist.tile([P, DIM], F16)
        nc.sync.dma_start(out=wout_sb, in_=wout_d)
        negc_sb = persist.tile([P, 1], F32)
        nc.vector.memset(negc_sb, -C_LOC)
        negm_sb = persist.tile([P, 1], F32)
        nc.vector.memset(negm_sb, -C_MEM)

        # ---- persistent activations (per-tile for fine-grained deps) --
        qhT_c = [persist.tile([P, 512], F16, name=f"qhT{i}") for i in range(nq)]
        kh2T_t = [persist.tile([P, P], F16, name=f"kh2T{i}") for i in range(nt)]
        qrow_t = [persist.tile([P, P], F16, name=f"qrow{i}") for i in range(nt)]
        vb_t = [persist.tile([P, DH + 1], BF16, name=f"vb{i}") for i in range(nt)]
        a_t = [persist.tile([P, P], F16, name=f"a{i}") for i in range(nt)]

        # phase-B stream pools opened BEFORE phase A so their addresses do
        # not alias phase-A tiles -> DMA prefetch overlaps the projections
        sb2 = ctx.enter_context(tc.tile_pool(name="sb2", bufs=2))
        sb3 = ctx.enter_context(tc.tile_pool(name="sb3", bufs=3))
        sc = ctx.enter_context(tc.tile_pool(name="sc", bufs=4))
        scm = ctx.enter_context(tc.tile_pool(name="scm", bufs=8))

        # ================= Phase A: projections ========================
        with ExitStack() as actx:
            pa = actx.enter_context(tc.tile_pool(name="pa", bufs=1))
            xt_sb = pa.tile([P, DIM // P, n], F16)
            xt_r = xt_d.rearrange("(c p) n -> p c n", p=P)
            for cc in range(DIM // P):
                nc.sync.dma_start(out=xt_sb[:, cc, :], in_=xt_r[:, cc, :])
            wq_sb = pa.tile([P, DIM // P, HPC * DH], F16)
            nc.sync.dma_start(out=wq_sb, in_=wq_d.rearrange("(c p) m -> p c m", p=P))
            wkv_sb = pa.tile([P, DIM // P, 2 * DH], F16)
            nc.sync.dma_start(out=wkv_sb, in_=wkv_d.rearrange("(c p) m -> p c m", p=P))

            qt_sb = pa.tile([P, n], F32)           # raw qT
            kvt_sb = pa.tile([P, n], F32)          # raw kT|vT

            with ExitStack() as pctx:
                psA = pctx.enter_context(tc.tile_pool(name="psA", bufs=1, space="PSUM"))
                q_ps = [psA.tile([P, 512], F32, tag=f"q{t}", name=f"q_ps{t}")
                        for t in range(nq)]
                kv_ps = [psA.tile([P, 512], F32, tag=f"kv{t}", name=f"kv_ps{t}")
                         for t in range(nq)]
                for c in range(DIM // P):
                    last = c == DIM // P - 1
                    for t in range(nq):
                        nc.tensor.matmul(q_ps[t], lhsT=wq_sb[:, c, :],
                                         rhs=xt_sb[:, c, bass.ts(t, 512)],
                                         start=(c == 0), stop=last)
                    for t in range(nq):
                        nc.tensor.matmul(kv_ps[t], lhsT=wkv_sb[:, c, :],
                                         rhs=xt_sb[:, c, bass.ts(t, 512)],
                                         start=(c == 0), stop=last)
                for t in range(nq):
                    nc.scalar.copy(qt_sb[:, bass.ts(t, 512)], q_ps[t])
                    nc.vector.tensor_copy(kvt_sb[:, bass.ts(t, 512)], kv_ps[t])

            # row-wise pass: normalize q (x scale) and k, build row tiles
            with ExitStack() as pctx:
                psR = pctx.enter_context(tc.tile_pool(name="psR", bufs=2, space="PSUM"))
                rsb = pctx.enter_context(tc.tile_pool(name="rsb", bufs=3))
                rsc = pctx.enter_context(tc.tile_pool(name="rsc", bufs=4))
                for t in range(nt):
                    qr_ps = psR.tile([P, P], F32, tag="qr")
                    nc.tensor.transpose(qr_ps, qt_sb[:, bass.ts(t, P)], id_f)
                    qr_sb = rsb.tile([P, P], F32, tag="qr_sb")
                    nc.scalar.copy(qr_sb, qr_ps)
                    kvr_ps = psR.tile([P, P], F32, tag="kvr")
                    nc.tensor.transpose(kvr_ps, kvt_sb[:, bass.ts(t, P)], id_f)
                    kvr_sb = rsb.tile([P, P], F32, tag="kvr_sb")
                    nc.scalar.copy(kvr_sb, kvr_ps)
                    nc.scalar.copy(vb_t[t][:, 0:DH], kvr_sb[:, DH:P])

                    sq_scr = rsb.tile([P, DH], F32, tag="sq_scr")
                    qhr = rsb.tile([P, P], F16, tag="qhr")
                    nall = rsc.tile([P, 4], F32, tag="nall")
                    nc.scalar.activation(out=sq_scr, in_=qr_sb[:, 0:DH],
                                         func=ACTF.Square, accum_out=nall[:, 0:1])
                    nc.scalar.activation(out=sq_scr, in_=qr_sb[:, DH:P],
                                         func=ACTF.Square, accum_out=nall[:, 1:2])
                    nc.scalar.activation(out=sq_scr, in_=kvr_sb[:, 0:DH],
                                         func=ACTF.Square, accum_out=nall[:, 2:3])
                    nc.scalar.sqrt(nall[:, 0:3], nall[:, 0:3])
                    nc.vector.reciprocal(nall[:, 0:3], nall[:, 0:3])
                    for h in range(HPC):
                        rq = rsc.tile([P, 1], F32, tag="rq")
                        nc.vector.tensor_tensor(out=rq, in0=nall[:, h:h + 1],
                                                in1=scales_sb[:, h:h + 1], op=OP.mult)
                        nc.vector.tensor_scalar_mul(qhr[:, bass.ts(h, DH)],
                                                    qr_sb[:, bass.ts(h, DH)], rq)
                    nc.scalar.copy(qrow_t[t], qhr)
                    qht_ps = psR.tile([P, P], F16, tag="qht")
                    nc.tensor.transpose(qht_ps, qhr, id_h)
                    nc.scalar.copy(qhT_c[t // 4][:, bass.ts(t % 4, P)], qht_ps)

                    khr = rsb.tile([P, DH], F16, tag="khr")
                    nc.vector.tensor_scalar_mul(khr, kvr_sb[:, 0:DH], nall[:, 2:3])
                    kht_ps = psR.tile([DH, P], F16, tag="kht")
                    nc.tensor.transpose(kht_ps, khr, id_h)
                    nc.scalar.copy(kh2T_t[t][0:DH, :], kht_ps)
                    nc.scalar.copy(kh2T_t[t][DH:P, :], kht_ps)
                    nc.vector.memset(vb_t[t][:, DH:DH + 1], 1.0)

        # ================= Phase B: attention ==========================
        with ExitStack() as bctx:
            sim_pool = bctx.enter_context(tc.tile_pool(name="simp", bufs=2, space="PSUM"))
            acc_pool = bctx.enter_context(tc.tile_pool(name="accp", bufs=1, space="PSUM"))
            pso = bctx.enter_context(tc.tile_pool(name="pso", bufs=1, space="PSUM"))

            for h in range(HPC):
                mask_h = sb2.tile([P, nt, KRET], F32, tag="mask", name=f"mask{h}")
                nc.sync.dma_start(out=mask_h,
                                  in_=mask_d[h].rearrange("(t p) j -> p t j", p=P))
                for c in range(nq):
                    # ---- memory branch for the 4 query tiles of chunk c --
                    mem_res = []
                    for g in range(4):
                        it = 4 * c + g
                        memk_t = sb2.tile([P, KRET, DH], F16, tag="memk")
                        nc.sync.dma_start(out=memk_t, in_=memk_d[h, bass.ts(it, P)])
                        memv_t = sb2.tile([P, DH + 1, KRET], BF16, tag="memv")
                        nc.sync.dma_start(out=memv_t, in_=memv_d[h, bass.ts(it, P)])

                        prodm = sb2.tile([P, KRET, DH], F32, tag="prodm")
                        nc.gpsimd.tensor_tensor(
                            out=prodm,
                            in0=bcast_mid(qrow_t[it][:, bass.ts(h, DH)], KRET),
                            in1=memk_t, op=OP.mult)
                        simm = sc.tile([P, KRET], F32, tag="simm")
                        nc.vector.tensor_reduce(out=simm, in_=prodm, axis=AX.X,
                                                op=OP.add)
                        nc.gpsimd.tensor_tensor(out=simm, in0=simm,
                                                in1=mask_h[:, it, :], op=OP.add)
                        wm = sc.tile([P, KRET], F32, tag="wm")
                        nc.scalar.activation(out=wm, in_=simm, func=ACTF.Exp, bias=negm_sb)
                        prodv = sb2.tile([P, DH + 1, KRET], F32, tag="prodv")
                        nc.gpsimd.tensor_tensor(out=prodv, in0=bcast_mid(wm, DH + 1),
                                                in1=memv_t, op=OP.mult)
                        mo = scm.tile([P, DH + 1], F32, tag="mo")
                        nc.vector.tensor_reduce(out=mo, in_=prodv, axis=AX.X,
                                                op=OP.add)
                        mem_res.append(mo)

                    # ---- local branch, transposed ------------------------
                    acc = [acc_pool.tile([P, DH + 1], F32, tag=f"acc{g}",
                                         name=f"acc{g}") for g in range(4)]
                    nkt = 4 * c + 4
                    expb_t = sb2.tile([P, 16, 512], BF16, tag="expb")
                    nc.sync.dma_start(
                        out=expb_t[:, 0:nkt, :],
                        in_=expb_d[h, c, 0:nkt * P, :].rearrange(
                            "(t p) q -> p t q", p=P))
                    for kt in range(nkt):
                        sim_ps = sim_pool.tile([P, 512], F32, tag="sim")
                        nc.tensor.matmul(sim_ps,
                                         lhsT=kh2T_t[kt][bass.ts(h, DH), :],
                                         rhs=qhT_c[c][bass.ts(h, DH), :],
                                         start=True, stop=True)
                        e_sb = sb3.tile([P, 512], BF16, tag="e")
                        nc.scalar.activation(out=e_sb, in_=sim_ps, func=ACTF.Exp,
                                             bias=negc_sb)
                        ebt = sb3.tile([P, 512], BF16, tag="ebt")
                        nc.gpsimd.tensor_tensor(out=ebt, in0=e_sb,
                                                in1=expb_t[:, kt, :], op=OP.mult)
                        for g in range(max(0, kt - 4 * c), 4):
                            it = 4 * c + g
                            nc.tensor.matmul(acc[g], lhsT=ebt[:, bass.ts(g, P)],
                                             rhs=vb_t[kt],
                                             start=(kt == 0), stop=(kt == it))

                    # ---- combine local + memory -------------------------
                    for g in range(4):
                        it = 4 * c + g
                        mo = mem_res[g]
                        slb = sc.tile([P, DH + 1], F32, tag="slb")
                        nc.vector.tensor_scalar_mul(slb, acc[g], B_LOC)
                        nc.vector.tensor_tensor(out=slb, in0=slb, in1=mo, op=OP.add)
                        rz = sc.tile([P, 1], F32, tag="rz")
                        nc.vector.reciprocal(rz, slb[:, DH:DH + 1])
                        nc.vector.tensor_scalar_mul(a_t[it][:, bass.ts(h, DH)],
                                                    slb[:, 0:DH], rz)
                        if h == HPC - 1:
                            # ---- output projection, interleaved ---------
                            at_ps = pso.tile([P, P], F16, tag="at")
                            nc.tensor.transpose(at_ps, a_t[it], id_h)
                            at_sb = sb2.tile([P, P], F16, tag="at_sb")
                            nc.scalar.copy(at_sb, at_ps)
                            o_ps = pso.tile([P, DIM], F32, tag="ops")
                            nc.tensor.matmul(o_ps, lhsT=at_sb, rhs=wout_sb,
                                             start=True, stop=True)
                            o_sb = sb2.tile([P, DIM], F16, tag="osb")
                            nc.scalar.copy(o_sb, o_ps)
                            nc.sync.dma_start(out=out_d[bass.ts(it, P), :], in_=o_sb)

    nc.compile()
    return nc


# ===================== host side =====================================

def prep_core_inputs(x, mem_kv, mem_mask, rel_pos_bias, Wq, Wkv, Wout,
                     scale_param):
    """Shard the full inputs into 8 per-core input maps."""
    b, n, dim = x.shape
    h = scale_param.shape[0]
    nq = n // 512
    bf = ml_dtypes.bfloat16

    scales = np.exp(np.asarray(scale_param, np.float32).reshape(h))
    xt = [np.ascontiguousarray(np.asarray(x[i], np.float32).T).astype(np.float16)
          for i in range(b)]
    expb = np.exp(np.asarray(rel_pos_bias[0], np.float32))
    iu = np.triu_indices(n, 1)
    expb[:, iu[0], iu[1]] = 0.0
    # transposed/blocked: expbT[h, c, j, i'] = expb[h, 512c+i', j]
    expbT = np.ascontiguousarray(
        expb.reshape(h, nq, 512, n).transpose(0, 1, 3, 2)).astype(bf)
    memk = np.asarray(mem_kv[..., 0, :], np.float32).astype(np.float16)
    memv_r = np.asarray(mem_kv[..., 1, :], np.float32).transpose(0, 1, 2, 4, 3)
    memv = np.empty(memv_r.shape[:3] + (memv_r.shape[3] + 1, memv_r.shape[4]), bf)
    memv[..., :-1, :] = memv_r.astype(bf)
    memv[..., -1, :] = np.asarray(1.0, bf)
    maskadd = np.where(np.asarray(mem_mask), 0.0, MASK_NEG).astype(np.float32)
    Wq16 = np.asarray(Wq, np.float32).astype(np.float16)
    Wkv16 = np.asarray(Wkv, np.float32).astype(np.float16)
    Wout16 = np.asarray(Wout, np.float32).astype(np.float16)

    in_maps = []
    for c in range(NCORES):
        bi, hg = divmod(c, NCORES // b)
        hs = slice(HPC * hg, HPC * hg + HPC)
        in_maps.append({
            "xt": xt[bi],
            "wq": np.ascontiguousarray(Wq16[:, HPC * DH * hg: HPC * DH * (hg + 1)]),
            "wkv": Wkv16,
            "wout": np.ascontiguousarray(Wout16[HPC * DH * hg: HPC * DH * (hg + 1), :]),
            "scales": np.ascontiguousarray(scales[hs]).reshape(1, HPC),
            "memk": np.ascontiguousarray(memk[bi, hs]),
            "memv": np.ascontiguousarray(memv[bi, hs]),
            "maskadd": np.ascontiguousarray(maskadd[bi, hs]),
            "expbias": np.ascontiguousarray(expbT[hs]),
        })
    return in_maps


_NC_CACHE = {}


def kernel(x, mem_kv, mem_mask, rel_pos_bias, Wq, Wkv, Wout, scale_param,
           trace=False):
    from concourse.bass_utils import run_bass_kernel_spmd

    b, n, dim = x.shape
    in_maps = prep_core_inputs(x, mem_kv, mem_mask, rel_pos_bias, Wq, Wkv,
                               Wout, scale_param)
    if n not in _NC_CACHE:
        _NC_CACHE[n] = build_nc(n)
    nc = _NC_CACHE[n]
    res = run_bass_kernel_spmd(nc, in_maps, core_ids=list(range(NCORES)),
                               trace=trace)
    outs = [r["out"] for r in res.results]
    full = np.zeros((b, n, dim), np.float32)
    g = NCORES // b
    for c in range(NCORES):
        full[c // g] += outs[c].astype(np.float32)
    if trace:
        kernel.last_results = res
    return full



# revision 8
# speedup vs baseline: 1.5850x; 1.5850x over previous
"""Trainium2 Bass kernel for nn_MemorizingTransformer (retrieval_knn).

Sharding: 8 cores = 2 batches x 4 head-pairs. Each core computes attention for
its batch and 2 heads plus its slice of the output projection; the host sums
the 4 partial outputs per batch (the "all-reduce after to_out").

Per-core algorithm (n=2048 tokens, dh=64, 2 heads, kret=32 memories):
  - q/k/v projected with weights stationary (fp16) -> qT/kT/vT in PSUM;
    a row-wise pass l2-normalizes q (folding in the logit scale) and k, and
    produces fp16 qhatT / khatT plus bf16 v rows.
  - local attention computed TRANSPOSED (keys on partitions): one matmul per
    (key-tile, query-chunk) gives simT; since qhat/khat are unit-scaled the
    logits are bounded by the scale, so exp uses a constant shift and its
    bf16 output is multiplied by a host-precomputed exp(rel_pos_bias) table
    (transposed/blocked, causal zeros) to give attention weights already in
    the [key, query] layout the PV matmul needs as its stationary operand.
  - an extra ones-column of v gives the local softmax partition sum for free.
  - memory attention runs on the TENSOR engine with a block-diagonal packing:
    queries are grouped 4 at a time so one group's 4x32 keys fill the 128
    contraction partitions.  Per group one matmul (free size 4) computes the
    4 queries' sims against the stacked keys into an E tile [128, (g,q')];
    exp + a {0,1} mask (zeroing cross-query terms) gives weights that drive
    per-group PV matmuls against host-packed values [128, 65] (65th col of
    ones = the partition sum), accumulating a transposed moT [65, 128] that
    one PE transpose restores to row-major.  mem_mask is folded into the
    packed values on the host (zeroed rows+ones ≡ -inf logits).
  - the two branches use different softmax shifts; the combine step rescales
    with a = exp(Cm-C*), b = exp(Cl-C*), C* = max(Cm, Cl) (both <= 1, no
    overflow/underflow blowups) before the output projection.
  - DMA traffic is spread over the three DMA-capable queues (SP, Activation,
    GpSimd) so transfers overlap; stream pools are allocated before phase-A
    pools so phase-B DMA prefetch overlaps the projections.
"""

import numpy as np
import ml_dtypes
from contextlib import ExitStack

import concourse.bass as bass
import concourse.bacc as bacc
import concourse.mybir as mybir
import concourse.tile as tile
from concourse.masks import make_identity

F32 = mybir.dt.float32
BF16 = mybir.dt.bfloat16
F16 = mybir.dt.float16
AX = mybir.AxisListType
OP = mybir.AluOpType
ACTF = mybir.ActivationFunctionType

P = 128
DIM = 512
DH = 64
KRET = 32
HPC = 2            # heads per core
NCORES = 8
C_LOC = 20.6       # >= scale * max|cos|: local exp args stay <= ~0
C_MEM = 60.0       # fixed mem-branch shift; mem logits ~N(0,20^2) stay
                   # well under exp overflow after -C_MEM (max arg ~ +45)
B_LOC = float(np.exp(C_LOC - C_MEM))   # rescale for local sums in combine


def bcast_mid(ap_2d, count):
    """[P, d] AP -> [P, count, d] AP broadcasting a new middle dim (step 0)."""
    return bass.AP(tensor=ap_2d.tensor, offset=ap_2d.offset,
                   ap=[list(ap_2d.ap[0]), [0, count], list(ap_2d.ap[1])])


def build_nc(n=2048):
    """Build the per-core Bass program (same NEFF for all 8 cores)."""
    nt = n // P               # 128-token tiles
    nq = n // 512             # 512-query chunks
    ng = P // 4               # 4-query groups per tile
    nc = bacc.Bacc("TRN2", target_bir_lowering=False, debug=False)

    xt_d = nc.dram_tensor("xt", (DIM, n), F16, kind="ExternalInput").ap()
    wq_d = nc.dram_tensor("wq", (DIM, HPC * DH), F16, kind="ExternalInput").ap()
    wkv_d = nc.dram_tensor("wkv", (DIM, 2 * DH), F16, kind="ExternalInput").ap()
    wout_d = nc.dram_tensor("wout", (HPC * DH, DIM), F16, kind="ExternalInput").ap()
    scales_d = nc.dram_tensor("scales", (1, HPC), F32, kind="ExternalInput").ap()
    # memkT[t, h*64+d, g*128+32j+k] = memk[h, 128t+4g+j, k, d]
    memkT_d = nc.dram_tensor("memkT", (nt, P, KRET * P), F16, kind="ExternalInput").ap()
    # memvp[t, 32j+k, h, g, d] = memv[h, 128t+4g+j, k, d]; [...,64] = mask
    memvp_d = nc.dram_tensor("memvp", (nt, P, HPC, KRET, DH + 1), BF16,
                             kind="ExternalInput").ap()
    # maskc[32j+k, j'] = 1 if j == j' else 0
    maskc_d = nc.dram_tensor("maskc", (P, 4), BF16, kind="ExternalInput").ap()
    # expbias[h, c, j, i'] = exp(bias[h, 512c+i', j]) * (512c+i' >= j)
    expb_d = nc.dram_tensor("expbias", (HPC, nq, n, 512), BF16, kind="ExternalInput").ap()
    out_d = nc.dram_tensor("out", (n, DIM), F16, kind="ExternalOutput").ap()

    with tile.TileContext(nc) as tc, ExitStack() as ctx:
        persist = ctx.enter_context(tc.tile_pool(name="persist", bufs=1))

        # ---- constants -------------------------------------------------
        id_f = persist.tile([P, P], F32)
        make_identity(nc, id_f)
        id_h = persist.tile([P, P], F16)
        make_identity(nc, id_h)
        id_b = persist.tile([P, P], BF16)
        make_identity(nc, id_b)
        scales_sb = persist.tile([P, HPC], F32)
        nc.sync.dma_start(out=scales_sb, in_=bass.AP(
            tensor=scales_d.tensor, offset=scales_d.offset,
            ap=[[0, P], list(scales_d.ap[1])]))
        wout_sb = persist.tile([P, DIM], F16)
        nc.sync.dma_start(out=wout_sb, in_=wout_d)
        maskc_sb = persist.tile([P, 4], BF16)
        nc.sync.dma_start(out=maskc_sb, in_=maskc_d)
        negc_sb = persist.tile([P, 1], F32)
        nc.vector.memset(negc_sb, -C_LOC)
        negm_sb = persist.tile([P, 1], F32)
        nc.vector.memset(negm_sb, -C_MEM)

        # ---- persistent activations (per-tile for fine-grained deps) --
        qhT_c = [persist.tile([P, 512], F16, name=f"qhT{i}") for i in range(nq)]
        kh2T_t = [persist.tile([P, P], F16, name=f"kh2T{i}") for i in range(nt)]
        vb_t = [persist.tile([P, DH + 1], BF16, name=f"vb{i}") for i in range(nt)]
        a_t = [persist.tile([P, P], F16, name=f"a{i}") for i in range(nt)]

        # phase-B stream pools opened BEFORE phase A so their addresses do
        # not alias phase-A tiles -> DMA prefetch overlaps the projections
        sbK = ctx.enter_context(tc.tile_pool(name="sbK", bufs=6))
        sbV = ctx.enter_context(tc.tile_pool(name="sbV", bufs=6))
        sb2 = ctx.enter_context(tc.tile_pool(name="sb2", bufs=2))
        sb3 = ctx.enter_context(tc.tile_pool(name="sb3", bufs=3))
        sc = ctx.enter_context(tc.tile_pool(name="sc", bufs=4))
        scm = ctx.enter_context(tc.tile_pool(name="scm", bufs=4))

        # ================= Phase A: projections ========================
        with ExitStack() as actx:
            pa = actx.enter_context(tc.tile_pool(name="pa", bufs=1))
            xt_sb = pa.tile([P, DIM // P, n], F16)
            xt_r = xt_d.rearrange("(c p) n -> p c n", p=P)
            for cc in range(DIM // P):
                nc.sync.dma_start(out=xt_sb[:, cc, :], in_=xt_r[:, cc, :])
            wq_sb = pa.tile([P, DIM // P, HPC * DH], F16)
            nc.sync.dma_start(out=wq_sb, in_=wq_d.rearrange("(c p) m -> p c m", p=P))
            wkv_sb = pa.tile([P, DIM // P, 2 * DH], F16)
            nc.sync.dma_start(out=wkv_sb, in_=wkv_d.rearrange("(c p) m -> p c m", p=P))

            qt_sb = pa.tile([P, n], F32)           # raw qT
            kvt_sb = pa.tile([P, n], F32)          # raw kT|vT

            with ExitStack() as pctx:
                psA = pctx.enter_context(tc.tile_pool(name="psA", bufs=1, space="PSUM"))
                q_ps = [psA.tile([P, 512], F32, tag=f"q{t}", name=f"q_ps{t}")
                        for t in range(nq)]
                kv_ps = [psA.tile([P, 512], F32, tag=f"kv{t}", name=f"kv_ps{t}")
                         for t in range(nq)]
                for c in range(DIM // P):
                    last = c == DIM // P - 1
                    for t in range(nq):
                        nc.tensor.matmul(q_ps[t], lhsT=wq_sb[:, c, :],
                                         rhs=xt_sb[:, c, bass.ts(t, 512)],
                                         start=(c == 0), stop=last)
                    for t in range(nq):
                        nc.tensor.matmul(kv_ps[t], lhsT=wkv_sb[:, c, :],
                                         rhs=xt_sb[:, c, bass.ts(t, 512)],
                                         start=(c == 0), stop=last)
                for t in range(nq):
                    nc.scalar.copy(qt_sb[:, bass.ts(t, 512)], q_ps[t])
                    nc.vector.tensor_copy(kvt_sb[:, bass.ts(t, 512)], kv_ps[t])

            # row-wise pass: normalize q (x scale) and k, build row tiles
            with ExitStack() as pctx:
                psR = pctx.enter_context(tc.tile_pool(name="psR", bufs=2, space="PSUM"))
                rsb = pctx.enter_context(tc.tile_pool(name="rsb", bufs=3))
                rsc = pctx.enter_context(tc.tile_pool(name="rsc", bufs=4))
                for t in range(nt):
                    qr_ps = psR.tile([P, P], F32, tag="qr")
                    nc.tensor.transpose(qr_ps, qt_sb[:, bass.ts(t, P)], id_f)
                    kvr_ps = psR.tile([P, P], F32, tag="kvr")
                    nc.tensor.transpose(kvr_ps, kvt_sb[:, bass.ts(t, P)], id_f)
                    nc.vector.tensor_copy(vb_t[t][:, 0:DH], kvr_ps[:, DH:P])

                    sq_scr = rsb.tile([P, DH], F32, tag="sq_scr")
                    qhr = rsb.tile([P, P], F16, tag="qhr")
                    nall = rsc.tile([P, 4], F32, tag="nall")
                    nc.scalar.activation(out=sq_scr, in_=qr_ps[:, 0:DH],
                                         func=ACTF.Square, accum_out=nall[:, 0:1])
                    nc.scalar.activation(out=sq_scr, in_=qr_ps[:, DH:P],
                                         func=ACTF.Square, accum_out=nall[:, 1:2])
                    nc.scalar.activation(out=sq_scr, in_=kvr_ps[:, 0:DH],
                                         func=ACTF.Square, accum_out=nall[:, 2:3])
                    nc.scalar.sqrt(nall[:, 0:3], nall[:, 0:3])
                    nc.vector.reciprocal(nall[:, 0:3], nall[:, 0:3])
                    for h in range(HPC):
                        rq = rsc.tile([P, 1], F32, tag="rq")
                        nc.vector.tensor_tensor(out=rq, in0=nall[:, h:h + 1],
                                                in1=scales_sb[:, h:h + 1], op=OP.mult)
                        nc.vector.tensor_scalar_mul(qhr[:, bass.ts(h, DH)],
                                                    qr_ps[:, bass.ts(h, DH)], rq)
                    qht_ps = psR.tile([P, P], F16, tag="qht")
                    nc.tensor.transpose(qht_ps, qhr, id_h)
                    nc.vector.tensor_copy(qhT_c[t // 4][:, bass.ts(t % 4, P)], qht_ps)

                    khr = rsb.tile([P, DH], F16, tag="khr")
                    nc.vector.tensor_scalar_mul(khr, kvr_ps[:, 0:DH], nall[:, 2:3])
                    kht_ps = psR.tile([DH, P], F16, tag="kht")
                    nc.tensor.transpose(kht_ps, khr, id_h)
                    nc.vector.tensor_copy(kh2T_t[t][0:DH, :], kht_ps)
                    nc.vector.tensor_copy(kh2T_t[t][DH:P, :], kht_ps)
                    nc.vector.memset(vb_t[t][:, DH:DH + 1], 1.0)

        # ================= Phase B: attention ==========================
        with ExitStack() as bctx:
            sim_pool = bctx.enter_context(tc.tile_pool(name="simp", bufs=2, space="PSUM"))
            acc_pool = bctx.enter_context(tc.tile_pool(name="accp", bufs=1, space="PSUM"))
            mem_ps = bctx.enter_context(tc.tile_pool(name="memps", bufs=1, space="PSUM"))
            eps_pool = mot_pool = mo_pool = mem_ps
            pso = bctx.enter_context(tc.tile_pool(name="pso", bufs=1, space="PSUM"))

            for c in range(nq):
                # ---- prefetch this chunk's memory tiles (both heads) ----
                memk_t = []
                memv_t = []
                for g in range(4):
                    it = 4 * c + g
                    mk = sbK.tile([P, KRET * P], F16, tag="memk", name=f"mk{it}")
                    nc.gpsimd.dma_start(out=mk, in_=memkT_d[it])
                    memk_t.append(mk)
                    mv = sbV.tile([P, HPC, KRET, DH + 1], BF16, tag="memv",
                                  name=f"mv{it}")
                    q_eng = nc.sync if g % 2 == 0 else nc.gpsimd
                    q_eng.dma_start(out=mv, in_=memvp_d[it])
                    memv_t.append(mv)

                for h in range(HPC):
                    hs = bass.ts(h, DH)
                    # ---- local branch, transposed ------------------------
                    acc_t = acc_pool.tile([P, 4, DH + 1], F32, tag="acc",
                                          name="acc")
                    acc = [acc_t[:, g, :] for g in range(4)]
                    nkt = 4 * c + 4
                    expb_t = sb2.tile([P, 16, 512], BF16, tag="expb")
                    nc.sync.dma_start(
                        out=expb_t[:, 0:nkt, :],
                        in_=expb_d[h, c, 0:nkt * P, :].rearrange(
                            "(t p) q -> p t q", p=P))
                    for kt in range(nkt):
                        sim_ps = sim_pool.tile([P, 512], F32, tag="sim")
                        nc.tensor.matmul(sim_ps,
                                         lhsT=kh2T_t[kt][hs, :],
                                         rhs=qhT_c[c][hs, :],
                                         start=True, stop=True)
                        e_sb = sb3.tile([P, 512], BF16, tag="e")
                        nc.scalar.activation(out=e_sb, in_=sim_ps, func=ACTF.Exp,
                                             bias=negc_sb)
                        ebt = sb3.tile([P, 512], BF16, tag="ebt")
                        eng = nc.gpsimd if kt % 2 == 0 else nc.vector
                        with nc.allow_low_precision(reason="attn weights bf16"):
                            eng.tensor_tensor(out=ebt, in0=e_sb,
                                              in1=expb_t[:, kt, :], op=OP.mult)
                        for g in range(max(0, kt - 4 * c), 4):
                            it = 4 * c + g
                            nc.tensor.matmul(acc[g], lhsT=ebt[:, bass.ts(g, P)],
                                             rhs=vb_t[kt],
                                             start=(kt == 0), stop=(kt == it))

                    # ---- memory branch on PE, block-diagonal -------------
                    for g in range(4):
                        it = 4 * c + g
                        E_ps = eps_pool.tile([P, ng, 4], F32, tag="E")
                        for g2 in range(ng):
                            nc.tensor.matmul(
                                E_ps[:, g2, :],
                                lhsT=memk_t[g][hs, bass.ts(g2, P)],
                                rhs=qhT_c[c][hs, g * P + 4 * g2:g * P + 4 * g2 + 4],
                                start=True, stop=True)
                        E_sb = scm.tile([P, ng, 4], BF16, tag="E_sb")
                        with nc.allow_low_precision(reason="mem weights bf16"):
                            nc.scalar.activation(out=E_sb, in_=E_ps, func=ACTF.Exp,
                                                 bias=negm_sb)
                            nc.vector.tensor_tensor(out=E_sb, in0=E_sb,
                                                    in1=bcast_mid(maskc_sb, ng),
                                                    op=OP.mult)
                        moT_ps = mot_pool.tile([DH + 1, P], F32, tag="moT")
                        for g2 in range(ng):
                            nc.tensor.matmul(moT_ps[:, bass.ts(g2, 4)],
                                             lhsT=memv_t[g][:, h, g2, :],
                                             rhs=E_sb[:, g2, :],
                                             start=True, stop=True)
                        moT_sb = scm.tile([DH + 1, P], BF16, tag="moT_sb")
                        with nc.allow_low_precision(reason="mem out bf16"):
                            nc.vector.tensor_copy(moT_sb, moT_ps)
                        mo_ps = mo_pool.tile([P, DH + 1], BF16, tag="mo")
                        nc.tensor.transpose(mo_ps, moT_sb,
                                            id_b[0:DH + 1, 0:DH + 1])

                        # ---- combine local + memory ----------------------
                        slb = sc.tile([P, DH + 1], F32, tag="slb")
                        nc.vector.scalar_tensor_tensor(
                            out=slb, in0=acc[g], scalar=B_LOC, in1=mo_ps,
                            op0=OP.mult, op1=OP.add)
                        rz = sc.tile([P, 1], F32, tag="rz")
                        nc.vector.reciprocal(rz, slb[:, DH:DH + 1])
                        nc.vector.tensor_scalar_mul(a_t[it][:, hs],
                                                    slb[:, 0:DH], rz)
                        if h == HPC - 1:
                            # ---- output projection, interleaved ---------
                            at_ps = pso.tile([P, P], F16, tag="at")
                            nc.tensor.transpose(at_ps, a_t[it], id_h)
                            at_sb = sb2.tile([P, P], F16, tag="at_sb")
                            nc.vector.tensor_copy(at_sb, at_ps)
                            o_ps = pso.tile([P, DIM], F32, tag="ops")
                            nc.tensor.matmul(o_ps, lhsT=at_sb, rhs=wout_sb,
                                             start=True, stop=True)
                            o_sb = sb2.tile([P, DIM], F16, tag="osb")
                            nc.scalar.copy(o_sb, o_ps)
                            nc.scalar.dma_start(out=out_d[bass.ts(it, P), :], in_=o_sb)

    nc.compile()
    return nc


# ===================== host side =====================================

def prep_core_inputs(x, mem_kv, mem_mask, rel_pos_bias, Wq, Wkv, Wout,
                     scale_param):
    """Shard the full inputs into 8 per-core input maps."""
    b, n, dim = x.shape
    h = scale_param.shape[0]
    nq = n // 512
    nt = n // P
    bf = ml_dtypes.bfloat16

    scales = np.exp(np.asarray(scale_param, np.float32).reshape(h))
    xt = [np.ascontiguousarray(np.asarray(x[i], np.float32).T).astype(np.float16)
          for i in range(b)]
    expb = np.exp(np.asarray(rel_pos_bias[0], np.float32))
    iu = np.triu_indices(n, 1)
    expb[:, iu[0], iu[1]] = 0.0
    # transposed/blocked: expbT[h, c, j, i'] = expb[h, 512c+i', j]
    expbT = np.ascontiguousarray(
        expb.reshape(h, nq, 512, n).transpose(0, 1, 3, 2)).astype(bf)

    memk = np.asarray(mem_kv[..., 0, :], np.float32)   # b h n k d
    memv = np.asarray(mem_kv[..., 1, :], np.float32)   # b h n k d
    mask = np.asarray(mem_mask)                        # b h n k

    # memkT[b, h, t, d, (g,j,k)] = memk[b, h, 128t+4g+j, k, d]
    mk5 = memk.reshape(b, h, nt, KRET, 4, KRET, DH)    # b h t g j k d
    memkT = np.ascontiguousarray(
        mk5.transpose(0, 1, 2, 6, 3, 4, 5)             # b h t d g j k
    ).reshape(b, h, nt, DH, KRET * P).astype(np.float16)

    # memvp[b, t, (j,k), h, g, d] = memv[b, h, 128t+4g+j, k, d] * mask
    mv5 = memv.reshape(b, h, nt, KRET, 4, KRET, DH)    # b h t g j k d
    mvp = np.empty((b, nt, 4, KRET, h, KRET, DH + 1), np.float32)  # b t j k h g d
    mvp[..., 0:DH] = mv5.transpose(0, 2, 4, 5, 1, 3, 6)
    mvp[..., DH] = 1.0
    if not mask.all():
        m5 = mask.reshape(b, h, nt, KRET, 4, KRET)     # b h t g j k
        mvp *= m5.transpose(0, 2, 4, 5, 1, 3)[..., None]
    memvp = np.ascontiguousarray(mvp.reshape(b, nt, P, h, KRET, DH + 1)).astype(bf)

    maskc = np.zeros((P, 4), bf)
    for j in range(4):
        maskc[j * KRET:(j + 1) * KRET, j] = 1.0

    Wq16 = np.asarray(Wq, np.float32).astype(np.float16)
    Wkv16 = np.asarray(Wkv, np.float32).astype(np.float16)
    Wout16 = np.asarray(Wout, np.float32).astype(np.float16)

    in_maps = []
    for c in range(NCORES):
        bi, hg = divmod(c, NCORES // b)
        hs = slice(HPC * hg, HPC * hg + HPC)
        # memkT per core: [nt, (h,d), 4096]
        mkc = np.ascontiguousarray(
            memkT[bi, hs].transpose(1, 0, 2, 3).reshape(nt, P, KRET * P))
        in_maps.append({
            "xt": xt[bi],
            "wq": np.ascontiguousarray(Wq16[:, HPC * DH * hg: HPC * DH * (hg + 1)]),
            "wkv": Wkv16,
            "wout": np.ascontiguousarray(Wout16[HPC * DH * hg: HPC * DH * (hg + 1), :]),
            "scales": np.ascontiguousarray(scales[hs]).reshape(1, HPC),
            "memkT": mkc,
            "memvp": np.ascontiguousarray(memvp[bi, :, :, hs]),
            "maskc": maskc,
            "expbias": np.ascontiguousarray(expbT[hs]),
        })
    return in_maps


_NC_CACHE = {}


def kernel(x, mem_kv, mem_mask, rel_pos_bias, Wq, Wkv, Wout, scale_param,
           trace=False):
    from concourse.bass_utils import run_bass_kernel_spmd

    b, n, dim = x.shape
    in_maps = prep_core_inputs(x, mem_kv, mem_mask, rel_pos_bias, Wq, Wkv,
                               Wout, scale_param)
    if n not in _NC_CACHE:
        _NC_CACHE[n] = build_nc(n)
    nc = _NC_CACHE[n]
    res = run_bass_kernel_spmd(nc, in_maps, core_ids=list(range(NCORES)),
                               trace=trace)
    outs = [r["out"] for r in res.results]
    full = np.zeros((b, n, dim), np.float32)
    g = NCORES // b
    for c in range(NCORES):
        full[c // g] += outs[c].astype(np.float32)
    if trace:
        kernel.last_results = res
    return full
